# revision 11
# baseline (speedup 1.0000x reference)
"""YOLO-style DetectionLoss on 8 Trainium2 NeuronCores (Bass/Tile), v2.

Pure data parallelism: batch 8192 -> 1024 per core; 1024*49 = 50176 cells
as 128 partitions x 392 cells. Engine split (HW-calibrated rates:
DVE ~1.4ns/elem 1x, 0.7 2x, ACT ~1.15, Pool ~2.4):

  Pool: first-touch diffs (dxy), doubled sum/min-width (s6d, mn2),
        a share of the class conf-mask.
  ACT : sqrt(wh) pair, |dcx| (with x4/S scale), 2*twh0 broadcast
        materialization, and ONE fused Square+accumulate per chunk over a
        combined pre-weighted tile SQT = [cwx | cwh | dcm | pcm | mdcl].
  DVE : bf16 overlap chain in f-major layout (2x/4x modes), IoU division
        (f32 reciprocal_approx_fast), responsible-box mask via the
        mx + (1-conf)*BIG trick, mask-muls, class sub + mask share.

Loss term weights (5, 1, 0.5, 1) are folded into the masks (sqrt(5),
sqrt(0.5)) so a single f32 accumulator column per chunk holds the full
weighted sum; the host just sums and divides by B.
"""

import numpy as np

import concourse.bacc as bacc
import concourse.mybir as mybir
import concourse.tile as tile
from concourse.bass_utils import run_bass_kernel_spmd

F32 = mybir.dt.float32
BF16 = mybir.dt.bfloat16
AF = mybir.ActivationFunctionType
OP = mybir.AluOpType
AX = mybir.AxisListType

NB, C, S = 3, 20, 7
D = 5 * NB + C                 # 35
B = 8192
NCORES = 8
P = 128

SQ5 = 5.0 ** 0.5
SQH = 0.5 ** 0.5
BIG = 1000.0

# class conf-mask channel split: [0:CLS_POOL) on Pool, rest on DVE
CLS_POOL = 11


def default_chunks(kpp):
    if kpp % 98 == 0:
        return [98] * (kpp // 98)
    return [kpp]


def build_nc(bc: int, ks=None, io_bufs: int = 3, loop_repeats: int = 0,
             cls_pool: int = CLS_POOL, repeats: int = 1, debug_sqt: bool = False):
    cells = bc * S * S
    assert cells % P == 0
    kpp = cells // P
    if ks is None:
        ks = default_chunks(kpp)
    assert sum(ks) == kpp
    nchunks = len(ks)

    nc = bacc.Bacc("TRN2", debug=False, num_devices=NCORES)
    out_h = nc.dram_tensor("output", [bc, S, S, D], F32, kind="ExternalInput")
    tgt_h = nc.dram_tensor("target", [bc, S, S, D], F32, kind="ExternalInput")
    acc_h = nc.dram_tensor("acc", [P, nchunks], F32, kind="ExternalOutput")
    sqt_h = (nc.dram_tensor("sqtd", [P, ks[0] * 38], F32, kind="ExternalOutput")
             if debug_sqt else None)

    out_v = out_h.ap().rearrange("(p a) h w d -> p (a h w d)", p=P)
    tgt_v = tgt_h.ap().rearrange("(p a) h w d -> p (a h w d)", p=P)

    with tile.TileContext(nc) as tc:
        with (
            tc.tile_pool(name="io", bufs=io_bufs) as io_pool,
            tc.tile_pool(name="wk", bufs=2) as wk,
            tc.tile_pool(name="accp", bufs=1) as accp,
        ):
            acc = accp.tile([P, nchunks], F32, name="acc")

            import contextlib
            loop_cm = (tc.For_i(0, loop_repeats, 1) if loop_repeats
                       else contextlib.nullcontext())
            with loop_cm:
              for _rep in range(repeats):
                off = 0
                for ci, k in enumerate(ks):
                    ot = io_pool.tile([P, k * D], F32, name="ot", tag="ot")
                    tt = io_pool.tile([P, k * D], F32, name="tt", tag="tt")
                    nc.sync.dma_start(ot[:], out_v[:, off:off + k * D])
                    nc.sync.dma_start(tt[:], tgt_v[:, off:off + k * D])
                    off += k * D

                    o3 = ot[:].rearrange("p (k d) -> p k d", d=D)
                    t3 = tt[:].rearrange("p (k d) -> p k d", d=D)
                    ob = o3[:, :, 0:15].rearrange("p k (b f) -> p k b f", f=5)
                    tb = t3[:, :, 0:15].rearrange("p k (b f) -> p k b f", f=5)

                    pxy = ob[:, :, :, 0:2]          # [P,k,3,2]
                    pwh = ob[:, :, :, 2:4]
                    pwhT = pwh.rearrange("p k b f -> p k f b")
                    twhT = tb[:, :, :, 2:4].rearrange("p k b f -> p k f b")
                    pcls = o3[:, :, 15:35]
                    tcls = t3[:, :, 15:35]
                    txy0 = t3[:, :, 0:2]            # [P,k,2]
                    twh0 = t3[:, :, 2:4]
                    confv = t3[:, :, 4]             # [P,k] 0/1

                    def WT(shape, dt, name):
                        return wk.tile(shape, dt, name=name, tag=name)[:]

                    # --- tiles ---
                    dxyf = WT([P, k, 2, 3], BF16, "dxyf")
                    t0wf = WT([P, k, 2, 3], BF16, "t0wf")
                    p2wf = WT([P, k, 2, 3], BF16, "p2wf")
                    s6d = WT([P, k, 2, 3], BF16, "s6d")
                    mn2 = WT([P, k, 2, 3], BF16, "mn2")
                    spf = WT([P, k, 2, 3], BF16, "spf")
                    stf = WT([P, k, 2, 3], BF16, "stf")
                    dwhf = WT([P, k, 2, 3], BF16, "dwhf")
                    dcx = WT([P, k, 3, 2], BF16, "dcx")
                    acxf = WT([P, k, 2, 3], BF16, "acxf")
                    u6 = WT([P, k, 2, 3], BF16, "u6")
                    ovf = WT([P, k, 2, 3], BF16, "ovf")
                    ovr = WT([P, k, 2, 3], BF16, "ovr")
                    rm5f = WT([P, k, 2, 3], BF16, "rm5f")
                    dclb = WT([P, k, 20], BF16, "dclb")
                    sqt = WT([P, k, 38], BF16, "sqt")
                    inter = WT([P, k, 3], BF16, "inter")
                    rmb = WT([P, k, 3], BF16, "rmb")
                    a1 = WT([P, k, 3], F32, "a1")
                    a12 = WT([P, k, 3], F32, "a12")
                    den = WT([P, k, 3], F32, "den")
                    rcp = WT([P, k, 3], F32, "rcp")
                    iou = WT([P, k, 3], F32, "iou")
                    dct = WT([P, k, 3], F32, "dct")
                    a2 = WT([P, k], F32, "a2")
                    cc = WT([P, k], F32, "cc")
                    mxh = WT([P, k], F32, "mxh")
                    mxc = WT([P, k], F32, "mxc")
                    mx = WT([P, k], F32, "mx")
                    notc = WT([P, k], F32, "notc")

                    cwx = sqt[:, :, 0:6].rearrange("p k (f b) -> p k f b", b=3)
                    cwh = sqt[:, :, 6:12].rearrange("p k (f b) -> p k f b", b=3)
                    sdcm = sqt[:, :, 12:15]
                    smdcl = sqt[:, :, 18:38]

                    # ---------- ACT: unary materializations ----------
                    nc.scalar.activation(
                        t0wf, twh0.unsqueeze(3).broadcast_to([P, k, 2, 3]),
                        AF.Copy, scale=2.0)
                    nc.scalar.activation(p2wf, pwhT, AF.Copy, scale=2.0)
                    nc.scalar.activation(spf, pwhT, AF.Sqrt)
                    nc.scalar.activation(stf, twhT, AF.Sqrt)

                    # ---------- Pool: first-touch ----------
                    nc.gpsimd.tensor_sub(
                        dxyf, pxy.rearrange("p k b f -> p k f b"),
                        tb[:, :, :, 0:2].rearrange("p k b f -> p k f b"))
                    nc.gpsimd.tensor_add(s6d, p2wf, t0wf)
                    nc.vector.tensor_tensor(mn2, p2wf, t0wf, op=OP.min)

                    # ---------- DVE: prep ----------
                    nc.vector.tensor_copy(cc, confv)
                    nc.vector.tensor_sub(
                        dcx, pxy,
                        txy0.unsqueeze(2).broadcast_to([P, k, 3, 2]))
                    nc.scalar.activation(
                        acxf, dcx.rearrange("p k b f -> p k f b"),
                        AF.Abs, scale=4.0 / S)

                    # class sub (bf16 out), split channel-wise DVE/Pool
                    cp = cls_pool
                    if cp > 0:
                        nc.gpsimd.tensor_sub(
                            dclb[:, :, 0:cp], pcls[:, :, 0:cp],
                            tcls[:, :, 0:cp])
                    if cp < 20:
                        nc.vector.tensor_sub(
                            dclb[:, :, cp:20], pcls[:, :, cp:20],
                            tcls[:, :, cp:20])

                    # ---------- DVE: overlap chain (bf16, f-major) ------
                    # u6 = 2(pw+tw) - 4|dc| = 4*(true half-sum overlap term)
                    # ov = min(2*mn2, u6) = 4*true overlap width
                    nc.vector.tensor_sub(u6, s6d, acxf)
                    nc.vector.scalar_tensor_tensor(
                        ovf, mn2, 2.0, u6, op0=OP.mult, op1=OP.min)
                    nc.vector.tensor_scalar_max(ovr, ovf, 0.0)
                    nc.vector.tensor_mul(inter, ovr[:, :, 0, :],
                                         ovr[:, :, 1, :])

                    # areas (x16 to match inter = 16*true), den, iou
                    nc.vector.scalar_tensor_tensor(
                        a1, ob[:, :, :, 2], 16.0, ob[:, :, :, 3],
                        op0=OP.mult, op1=OP.mult)
                    nc.vector.scalar_tensor_tensor(
                        a2, twh0[:, :, 0], 16.0, twh0[:, :, 1],
                        op0=OP.mult, op1=OP.mult)
                    nc.vector.tensor_add(
                        a12, a1, a2.unsqueeze(2).broadcast_to([P, k, 3]))
                    nc.vector.tensor_sub(den, a12, inter)
                    nc.vector.reciprocal_approx_fast(
                        rcp.rearrange("p k b -> p (k b)"),
                        den.rearrange("p k b -> p (k b)"))
                    nc.vector.tensor_mul(iou, inter, rcp)

                    # responsible-box mask: rm = (iou >= mx + (cc!=1)*BIG).
                    # (cc!=1)*BIG is exactly 0 for obj cells, so mxc == mx
                    # bit-exactly there.
                    nc.vector.tensor_reduce(mx, iou, axis=AX.X, op=OP.max)
                    nc.vector.tensor_scalar(
                        mxh, cc, 1.0, BIG, op0=OP.not_equal, op1=OP.mult)
                    nc.vector.tensor_add(mxc, mx, mxh)
                    for b in range(3):
                        nc.vector.tensor_tensor(
                            rmb[:, :, b], iou[:, :, b], mxc, op=OP.is_ge)
                    nc.vector.tensor_scalar_mul(
                        rm5f, rmb.unsqueeze(2).broadcast_to([P, k, 2, 3]),
                        SQ5)

                    # ---------- masked residuals into SQT ----------
                    nc.vector.tensor_mul(cwx, dxyf, rm5f)
                    nc.vector.tensor_sub(dwhf, spf, stf)
                    nc.vector.tensor_mul(cwh, dwhf, rm5f)
                    for b in range(3):
                        nc.vector.tensor_sub(dct[:, :, b], ob[:, :, b, 4], mx)
                    nc.vector.tensor_mul(sdcm, dct, rmb)
                    nc.vector.tensor_scalar(
                        notc, cc, 1.0, SQH, op0=OP.not_equal, op1=OP.mult)
                    for b in range(3):
                        nc.vector.tensor_mul(sqt[:, :, 15 + b],
                                             ob[:, :, b, 4], notc)
                    # class conf-mask, split channel-wise
                    ccb20 = cc.unsqueeze(2)
                    if cp > 0:
                        nc.gpsimd.tensor_mul(
                            smdcl[:, :, 0:cp], dclb[:, :, 0:cp],
                            ccb20.broadcast_to([P, k, cp]))
                    if cp < 20:
                        nc.vector.tensor_mul(
                            smdcl[:, :, cp:20], dclb[:, :, cp:20],
                            ccb20.broadcast_to([P, k, 20 - cp]))

                    if debug_sqt and ci == 0:
                        sq32 = WT([P, k, 38], F32, "sq32")
                        nc.vector.tensor_copy(sq32, sqt)
                        nc.sync.dma_start(
                            sqt_h.ap()[:],
                            sq32.rearrange("p k d -> p (k d)"))
                    # ---------- ACT: single fused Square+accumulate ----
                    nc.scalar.activation(sqt, sqt, AF.Square,
                                         accum_out=acc[:, ci:ci + 1])

            nc.sync.dma_start(acc_h.ap()[:], acc[:])

    nc.compile()
    return nc


_CACHE = {}


def _get_nc(bc, ks=None, io_bufs=3, loop_repeats=0, cls_pool=CLS_POOL,
            repeats=1, **_ignored):
    key = (bc, tuple(ks) if ks else None, io_bufs, loop_repeats, cls_pool,
           repeats)
    if key not in _CACHE:
        _CACHE[key] = build_nc(bc, ks, io_bufs, loop_repeats, cls_pool,
                               repeats)
    return _CACHE[key]


def combine_acc(acc_list, nchunks):
    """Host-side gather: weighted term sums are pre-folded on device; just
    sum everything and divide by the global batch."""
    tot = np.float64(0.0)
    for a in acc_list:
        tot += a.astype(np.float64).sum()
    return np.float32(tot / B)


BEST_KS = [98, 98, 98, 98]
BEST_IO_BUFS = 3


def extra_inputs():
    return {}


def kernel(output: np.ndarray, target: np.ndarray) -> np.ndarray:
    assert output.shape == (B, S, S, D) and target.shape == (B, S, S, D)
    bc = B // NCORES
    nchunks = len(BEST_KS)
    nc = _get_nc(bc, BEST_KS, io_bufs=BEST_IO_BUFS)
    in_maps = [
        {
            "output": np.ascontiguousarray(output[i * bc:(i + 1) * bc]),
            "target": np.ascontiguousarray(target[i * bc:(i + 1) * bc]),
        }
        for i in range(NCORES)
    ]
    res = run_bass_kernel_spmd(nc, in_maps, list(range(NCORES)))
    return combine_acc([r["acc"] for r in res.results], nchunks)
